# revision 13
# baseline (speedup 1.0000x reference)
"""Trainium2 Bass kernel v9 for nn_CorrectorEGNN (B=128 graphs, N=64, H=128, L=4).

v6 + single-matmul stage1: dst-major 512-col chunks touch only 8 dst blocks,
so one K=128 select matrix packs src one-hots (rows 0-63), the dynamic
p_i*p_j rows (64-66, written in-place by Pool each step), and dst one-hots
for blocks 0..60 (rows 67-127).  Blocks 61-63 get their B' term from a tiny
K=3 patch matmul on the last 192 columns.  This removes the 2x-cost psum
accumulate pairs that dominated stage1.

Diagonal-edge MLP (md) computed one tick early; its subtraction is folded
into the node MLP as a third accumulating matmul with negated nw1b.
"""

import sys

sys.path.insert(0, "/opt/trn_rl_repo")

import numpy as np

N = 64
C = 3
H = 128
L = 4
B = 128
NCORES = 8
GPC = B // NCORES
E = N * N

_CACHE = {}


def _prep_consts(inputs):
    f32 = np.float32
    ew1 = np.asarray(inputs["edge_w1"], f32)
    d = {}
    d["w1ab"] = np.concatenate(
        [np.concatenate([ew1[l, :H], ew1[l, H : 2 * H]], axis=1) for l in range(L)],
        axis=1,
    )
    wrow = ew1[:, 2 * H]
    d["wrep"] = np.concatenate(
        [np.tile(wrow[l][None, :], (N, 1)) for l in range(L)], axis=1
    )
    d["w3n"] = np.concatenate(
        [np.tile((-2.0 * wrow[l])[None, :], (C, 1)) for l in range(L)], axis=1
    )
    d["w2"] = np.concatenate([np.asarray(inputs["edge_w2"], f32)[l] for l in range(L)], axis=1)
    d["cw1"] = np.concatenate([np.asarray(inputs["coord_w1"], f32)[l] for l in range(L)], axis=1)
    d["cw2c"] = np.concatenate([np.asarray(inputs["coord_w2"], f32)[l] for l in range(L)], axis=1)
    nw1 = np.asarray(inputs["node_w1"], f32)
    d["nw1a"] = np.concatenate([nw1[l, :H] for l in range(L)], axis=1)
    d["nw1b"] = np.concatenate([nw1[l, H:] for l in range(L)], axis=1)
    d["nw1bn"] = -d["nw1b"]
    d["nw2"] = np.concatenate([np.asarray(inputs["node_w2"], f32)[l] for l in range(L)], axis=1)
    bias_cols = []
    for nm in ("edge_b1", "edge_b2", "coord_b1", "node_b1", "node_b2"):
        arr = np.asarray(inputs[nm], f32)
        for l in range(L):
            bias_cols.append(arr[l][:, None])
    d["biases"] = np.concatenate(bias_cols, axis=1)
    d["nerep"] = np.tile(np.asarray(inputs["node_embed"], f32).T, (1, N))
    d["ident"] = np.eye(N, dtype=f32)
    os_val = float(np.asarray(inputs["output_scale"], f32)[0])
    msc = np.zeros((N, 2), f32)
    msc[:, 0] = 1.0
    msc[:, 1] = os_val
    d["msc"] = msc
    d["inv64"] = np.full((1, N), 1.0 / N, f32)
    # unified stage1 select matrix, dst-major e = j*64+i:
    # rows 0-63 src one-hot; rows 64-66 zero (PP written at runtime);
    # rows 67-127 dst one-hot for j=0..60
    S = np.zeros((2 * N, E), f32)
    ee = np.arange(E)
    S[ee % N, ee] = 1.0
    jj = ee // N
    m61 = jj < 61
    S[67 + jj[m61], ee[m61]] = 1.0
    d["s128"] = S
    # patch for dst blocks 61-63 (last 192 columns)
    Sb3 = np.zeros((C, 3 * N), f32)
    cc = np.arange(3 * N)
    Sb3[cc // N, cc] = 1.0
    d["sb3"] = Sb3
    Sd = np.zeros((2 * N, N), f32)
    nn = np.arange(N)
    Sd[nn, nn] = 1.0
    Sd[N + nn, nn] = 1.0
    d["Sdiag"] = Sd
    return d


def _build(n_graphs, num_devices):
    import concourse.bacc as bacc
    import concourse.tile as tile
    import concourse.mybir as mybir
    from contextlib import ExitStack

    dt = mybir.dt
    f32 = dt.float32
    f32r = dt.float32r
    Silu = mybir.ActivationFunctionType.Silu
    add = mybir.AluOpType.add
    sub = mybir.AluOpType.subtract
    mult = mybir.AluOpType.mult
    AX = mybir.AxisListType.X

    nc = bacc.Bacc("TRN2", num_devices=num_devices, enable_partition_id=False)

    dr = {}
    for name, shape in [
        ("xin", [n_graphs, N, C]),
        ("xtin", [n_graphs, C, N]),
        ("s128", [2 * N, E]),
        ("sb3", [C, 3 * N]),
        ("Sdiag", [2 * N, N]),
        ("w1ab", [H, L * 2 * H]),
        ("wrep", [N, L * H]),
        ("w3n", [C, L * H]),
        ("w2", [H, L * H]),
        ("cw1", [H, L * H]),
        ("cw2c", [H, L]),
        ("nw1a", [H, L * H]),
        ("nw1b", [H, L * H]),
        ("nw1bn", [H, L * H]),
        ("nw2", [H, L * H]),
        ("biases", [H, 5 * L]),
        ("nerep", [H, N]),
        ("ident", [N, N]),
        ("msc", [N, 2]),
        ("inv64", [1, N]),
    ]:
        dr[name] = nc.dram_tensor(name, shape, f32, kind="ExternalInput").ap()
    y = nc.dram_tensor("y", [n_graphs, N, C], f32, kind="ExternalOutput").ap()

    F32R_CONSTS = {"s128", "sb3", "Sdiag", "w1ab", "w3n", "w2", "cw1", "cw2c",
                   "nw1a", "nw1b", "nw1bn", "nw2"}

    with nc.allow_low_precision(reason="fp32r matmul inputs"), tile.TileContext(nc) as tc, ExitStack() as es:
        cp = es.enter_context(tc.tile_pool(name="const", bufs=1))
        sp = es.enter_context(tc.tile_pool(name="state", bufs=1))
        wp = es.enter_context(tc.tile_pool(name="work", bufs=2))
        pbig = es.enter_context(tc.tile_pool(name="pbig", bufs=3, space="PSUM"))
        psm = es.enter_context(tc.tile_pool(name="psm", bufs=2, space="PSUM"))

        ct = {}
        for name in ("sb3", "Sdiag", "w1ab", "wrep", "w3n", "w2", "cw1", "cw2c",
                     "nw1a", "nw1b", "nw1bn", "nw2", "biases", "nerep", "ident",
                     "msc", "inv64"):
            cdt = f32r if name in F32R_CONSTS else f32
            t = cp.tile(list(dr[name].shape), cdt, tag=f"c_{name}", name=f"c_{name}")
            nc.sync.dma_start(out=t, in_=dr[name].bitcast(cdt) if cdt is f32r else dr[name])
            ct[name] = t

        def wsl(name, l):
            return ct[name][:, l * H : (l + 1) * H]

        def bsl(bi, l):
            return ct["biases"][:, bi * L + l : bi * L + l + 1]

        HTs, Pxs, PTs = [], [], []
        for g in range(n_graphs):
            HT = sp.tile([H, 2 * N], f32r, tag=f"HT{g}", name=f"HT{g}")
            nc.vector.tensor_copy(out=HT[:, 0:N], in_=ct["nerep"][:].bitcast(f32r))
            PxA = sp.tile([N, 4], f32, tag=f"PxA{g}", name=f"PxA{g}")
            P0 = sp.tile([N, C], f32, tag=f"P0{g}", name=f"P0{g}")
            nc.sync.dma_start(out=PxA[:, 0:3], in_=dr["xin"][g])
            nc.sync.dma_start(out=P0[:], in_=dr["xin"][g])
            nc.vector.memset(PxA[:, 3:4], 1.0)
            Px = [PxA, None, P0]
            PT = sp.tile([C, N], f32r, tag=f"PT{g}", name=f"PT{g}")
            nc.sync.dma_start(out=PT[:], in_=dr["xtin"][g].bitcast(f32r))
            HTs.append(HT)
            Pxs.append(Px)
            PTs.append(PT)
        for g in range(n_graphs):
            PxB = sp.tile([N, 4], f32, tag=f"PxB{g}", name=f"PxB{g}")
            nc.vector.memset(PxB[:, 3:4], 1.0)
            Pxs[g][1] = PxB

        # two rotating select-matrix slots; const rows loaded once into both
        s128_slots = []
        for si in range(2):
            st = wp.tile([2 * N, E], f32r, tag="s128", name=f"s128_{si}")
            nc.sync.dma_start(out=st, in_=dr["s128"].bitcast(f32r))
            s128_slots.append(st)

        def ptile(g, l):  # position tile holding the state entering layer l
            return Pxs[g][l % 2]

        def hcol(l):  # column of the h state entering layer l
            return N * (l % 2)

        lSs = [None] * n_graphs      # [A'(64); B'(64)] per graph (for md)
        gdTs = [None] * n_graphs     # |p|^2 as a row, for the md diag correction
        lS128s = [None] * n_graphs   # packed stage1 lhsT per graph
        B3s = [None] * n_graphs      # B'[61:64] per graph

        def build_ls(g, l, ab_ap):
            """lSfull + packed lS128 + B3 for (g, l)."""
            px = ptile(g, l)
            sq = wp.tile([N, C], f32, tag="sq", name="sq")
            nc.vector.tensor_tensor(out=sq[:], in0=px[:, 0:3], in1=px[:, 0:3], op=mult)
            gd = wp.tile([N, 1], f32, tag="gd", name="gd")
            nc.vector.tensor_reduce(out=gd[:], in_=sq[:], axis=AX, op=add)
            lS = sp.tile([2 * N, H], f32r, tag=f"lS{g}", name=f"lS{g}", bufs=1)
            wr = ct["wrep"][:, l * H : (l + 1) * H]
            nc.vector.scalar_tensor_tensor(out=lS[0:N, :], in0=wr, scalar=gd[:],
                                           in1=ab_ap[:, 0:H], op0=mult, op1=add)
            nc.vector.scalar_tensor_tensor(out=lS[N:, :], in0=wr, scalar=gd[:],
                                           in1=ab_ap[:, H:], op0=mult, op1=add)
            lSs[g] = lS
            gdT = sp.tile([1, N], f32r, tag=f"gdT{g}", name=f"gdT{g}", bufs=1)
            nc.gpsimd.dma_start(out=gdT[:], in_=gd[:].bitcast(f32r))
            gdTs[g] = gdT
            lX = sp.tile([2 * N, H], f32r, tag=f"lX{g}", name=f"lX{g}", bufs=1)
            nc.vector.tensor_copy(out=lX[0:N, :], in_=lS[0:N, :])
            nc.vector.tensor_copy(out=lX[64:67, :], in_=ct["w3n"][:, l * H : (l + 1) * H])
            nc.gpsimd.dma_start(out=lX[67:128, :], in_=lS[64:125, :])
            b3 = sp.tile([C, H], f32r, tag=f"b3{g}", name=f"b3{g}", bufs=1)
            nc.gpsimd.dma_start(out=b3[:], in_=lS[125:128, :])
            lS128s[g] = lX
            B3s[g] = b3

        def build_pp(g, slot_idx):
            """Write PP rows into select-matrix slot; also ppd (diag squares)."""
            st = s128_slots[slot_idx]
            Pv = st[64:67, :].rearrange("p (j i) -> p j i", j=N, i=N)
            pt = PTs[g]
            in0 = pt[:].unsqueeze(1).to_broadcast([C, N, N])
            in1 = pt[:].unsqueeze(2).to_broadcast([C, N, N])
            nc.gpsimd.tensor_tensor(out=Pv, in0=in0, in1=in1, op=mult)
            return (st,)

        def silu(out_ap, in_ap, bias_ap):
            nc.scalar.activation(out=out_ap, in_=in_ap, func=Silu, bias=bias_ap)

        def stage1_chunk(k_step, st, t1, c):
            g, l = k_step
            lX = lS128s[g]
            b3 = B3s[g]
            bt = pbig.tile([H, 1024], f32, tag="big", name="bt1")
            for q in range(2):
                sl = slice(c * 1024 + q * 512, c * 1024 + (q + 1) * 512)
                po = slice(q * 512, (q + 1) * 512)
                last = c == 3 and q == 1
                nc.tensor.matmul(out=bt[:, po], lhsT=lX[:], rhs=st[:, sl],
                                 start=True, stop=not last)
                if last:
                    nc.tensor.matmul(out=bt[:, 832:1024], lhsT=b3[:],
                                     rhs=ct["sb3"][:], start=False, stop=True)
            silu(t1[:, c * 1024 : (c + 1) * 1024], bt[:], bsl(0, l))

        def md_chain(k_step):
            """Diagonal-edge MLP md (one tick ahead of its node use)."""
            g, l = k_step
            lst = lSs[g]
            dz1 = psm.tile([H, N], f32, tag="sm", name="dz1")
            nc.tensor.matmul(out=dz1[:], lhsT=lst[:], rhs=ct["Sdiag"][:],
                             start=True, stop=False)
            nc.tensor.matmul(out=dz1[:], lhsT=ct["w3n"][0:1, l * H : (l + 1) * H],
                             rhs=gdTs[g][:], start=False, stop=True)
            t1d = wp.tile([H, N], f32r, tag="t1d", name="t1d", bufs=2)
            silu(t1d[:], dz1[:], bsl(0, l))
            dz2 = psm.tile([H, N], f32, tag="sm", name="dz2")
            nc.tensor.matmul(out=dz2[:], lhsT=wsl("w2", l), rhs=t1d[:],
                             start=True, stop=True)
            md = wp.tile([H, N], f32r, tag="md", name="md", bufs=2)
            silu(md[:], dz2[:], bsl(1, l))
            return md

        def node_chain(k_step, msg, md):
            """node MLP + h update (md subtraction folded in)."""
            g, l = k_step
            HT = HTs[g]
            ha, hb = hcol(l), hcol(l + 1)
            nps = psm.tile([H, N], f32, tag="sm", name="nps")
            nc.tensor.matmul(out=nps[:], lhsT=wsl("nw1a", l), rhs=HT[:, ha : ha + N],
                             start=True, stop=False)
            nc.tensor.matmul(out=nps[:], lhsT=wsl("nw1b", l), rhs=msg[:],
                             start=False, stop=False)
            nc.tensor.matmul(out=nps[:], lhsT=wsl("nw1bn", l), rhs=md[:],
                             start=False, stop=True)
            u = wp.tile([H, N], f32r, tag="u", name="u", bufs=2)
            silu(u[:], nps[:], bsl(3, l))
            nps2 = psm.tile([H, N], f32, tag="sm", name="nps2")
            nc.tensor.matmul(out=nps2[:], lhsT=wsl("nw2", l), rhs=u[:],
                             start=True, stop=True)
            nc.vector.scalar_tensor_tensor(out=HT[:, hb : hb + N], in0=nps2[:],
                                           scalar=bsl(4, l),
                                           in1=HT[:, ha : ha + N], op0=add, op1=add)

        def tailA(prev):
            pg, pl, pCWT = prev
            pa = ptile(pg, pl)
            pb = ptile(pg, pl + 1)
            upd = psm.tile([N, 4], f32, tag="sm", name="upd")
            nc.tensor.matmul(out=upd[:], lhsT=pCWT[:], rhs=pa[:, 0:4],
                             start=True, stop=True)
            tmpP = wp.tile([N, C], f32, tag="tmpP", name="tmpP")
            nc.vector.scalar_tensor_tensor(out=tmpP[:], in0=pa[:, 0:3],
                                           scalar=upd[:, 3:4], in1=upd[:, 0:3],
                                           op0=mult, op1=sub)
            nc.vector.tensor_tensor(out=pb[:, 0:3], in0=pa[:, 0:3],
                                    in1=tmpP[:], op=sub)
            return (pg, pl)

        def tailB(pend):
            pg, pl = pend
            pb_t = ptile(pg, pl + 1)
            pPT = PTs[pg]
            ptp = psm.tile([C, N], f32, tag="sm", name="ptp")
            nc.tensor.transpose(out=ptp[:], in_=pb_t[:, 0:3], identity=ct["ident"][:])
            nc.vector.tensor_copy(out=pPT[:], in_=ptp[:])
            if pl < L - 1:
                hc = hcol(pl + 1)
                ab = psm.tile([N, 2 * H], f32, tag="sm", name="ab")
                nc.tensor.matmul(out=ab[:], lhsT=HTs[pg][:, hc : hc + N],
                                 rhs=ct["w1ab"][:, (pl + 1) * 2 * H : (pl + 2) * 2 * H],
                                 start=True, stop=True)
                build_ls(pg, pl + 1, ab)
            else:
                dxt = wp.tile([N, C], f32, tag="dxt", name="dxt")
                nc.vector.tensor_tensor(out=dxt[:], in0=pb_t[:, 0:3],
                                        in1=Pxs[pg][2][:], op=sub)
                mps = psm.tile([1, C], f32, tag="sm", name="mps")
                nc.tensor.matmul(out=mps[:], lhsT=ct["msc"][:, 0:1], rhs=dxt[:],
                                 start=True, stop=True)
                means = wp.tile([1, C], f32, tag="means", name="means")
                nc.vector.tensor_copy(out=means[:], in_=mps[:])
                mrep = psm.tile([N, C], f32, tag="sm", name="mrep")
                nc.tensor.matmul(out=mrep[:], lhsT=ct["inv64"][:], rhs=means[:],
                                 start=True, stop=True)
                dx2 = wp.tile([N, C], f32, tag="dx2", name="dx2")
                nc.vector.tensor_tensor(out=dx2[:], in0=dxt[:], in1=mrep[:], op=sub)
                dx3 = wp.tile([N, C], f32, tag="dx3", name="dx3")
                nc.vector.tensor_scalar_mul(out=dx3[:], in0=dx2[:], scalar1=ct["msc"][:, 1:2])
                nc.sync.dma_start(out=y[pg], in_=dx3[:])

        # ---- startup ----
        ab0 = psm.tile([N, 2 * H], f32, tag="sm", name="ab0")
        nc.tensor.matmul(out=ab0[:], lhsT=HTs[0][:, 0:N], rhs=ct["w1ab"][:, 0 : 2 * H],
                         start=True, stop=True)
        for g in range(n_graphs):
            build_ls(g, 0, ab0)

        steps = [(g, l) for l in range(L) for g in range(n_graphs)]
        NS = len(steps)
        pps = {0: build_pp(steps[0][0], 0), 1: build_pp(steps[1][0], 1)}
        prev = None        # awaiting tailA (pos update)
        pendB = None       # awaiting tailB
        prev_cwji = None   # awaiting transpose
        mds = {}           # per-step md tiles
        ems = {}           # per-step m tiles

        t1_cur = wp.tile([H, E], f32r, tag="t1", name="t1", bufs=2)
        for c in range(4):
            stage1_chunk(steps[0], pps[0][0], t1_cur, c)

        for k in range(NS + 1):
            in_range = k < NS
            if in_range:
                g, l = steps[k]
                t1 = t1_cur

            if k + 2 < NS:
                pps[k + 2] = build_pp(steps[k + 2][0], k % 2)

            # CWJI -> CWT transpose (step k-2)
            if prev_cwji is not None:
                cg, cl, cwji = prev_cwji
                CWT = wp.tile([N, N], f32, tag="CWT", name="CWT", bufs=2)
                for bi in range(2):
                    for bj in range(2):
                        nc.vector.transpose(out=CWT[bj * 32 : bj * 32 + 32, bi * 32 : bi * 32 + 32],
                                            in_=cwji[bi * 32 : bi * 32 + 32, bj * 32 : bj * 32 + 32])
                prev = (cg, cl, CWT)
                prev_cwji = None

            # early msg reduce for step k-1 (DVE gets a head start)
            msg_prev = None
            if k - 1 >= 0 and k - 1 in mds:
                pg1, pl1 = steps[k - 1]
                msg_prev = wp.tile([H, N], f32r, tag="msg", name="msg", bufs=2)
                nc.vector.tensor_reduce(out=msg_prev[:],
                                        in_=ems[k - 1][:].rearrange("p (j i) -> p j i", j=N, i=N),
                                        axis=AX, op=add)

            if in_range and k + 1 < NS:
                t1_cur = wp.tile([H, E], f32r, tag="t1", name="t1", bufs=2)
            if in_range:
                em = wp.tile([H, E], f32r, tag="m", name="em", bufs=2)
            if k - 1 >= 0:
                t2 = wp.tile([H, E], f32r, tag="t2", name="t2", bufs=1)
                cwrA = wp.tile([97, 512], f32, tag="cwrA", name="cwrA", bufs=2)
                cwrB = wp.tile([97, 512], f32, tag="cwrB", name="cwrB", bufs=2)
                pg, pl = steps[k - 1]
                emp = ems[k - 1]

            for c in range(4):
                if in_range:
                    bt = pbig.tile([H, 1024], f32, tag="big", name="bt2")
                    for q in range(2):
                        sl = slice(c * 1024 + q * 512, c * 1024 + (q + 1) * 512)
                        po = slice(q * 512, (q + 1) * 512)
                        nc.tensor.matmul(out=bt[:, po], lhsT=wsl("w2", l), rhs=t1[:, sl],
                                         start=True, stop=True)
                    silu(em[:, c * 1024 : (c + 1) * 1024], bt[:], bsl(1, l))
                if in_range and k + 1 < NS:
                    stage1_chunk(steps[k + 1], pps[k + 1][0], t1_cur, c)
                if k - 1 >= 0:
                    bt3 = pbig.tile([H, 1024], f32, tag="big", name="bt3")
                    for q in range(2):
                        sl = slice(c * 1024 + q * 512, c * 1024 + (q + 1) * 512)
                        po = slice(q * 512, (q + 1) * 512)
                        nc.tensor.matmul(out=bt3[:, po], lhsT=wsl("cw1", pl), rhs=emp[:, sl],
                                         start=True, stop=True)
                    silu(t2[:, c * 1024 : (c + 1) * 1024], bt3[:], bsl(2, pl))
                    for q in range(2):
                        c8 = 2 * c + q
                        cwps = pbig.tile([1, 512], f32, tag="big", name="cwps")
                        nc.tensor.matmul(out=cwps[:], lhsT=ct["cw2c"][:, pl : pl + 1],
                                         rhs=t2[:, c8 * 512 : (c8 + 1) * 512],
                                         start=True, stop=True)
                        tgt = cwrA if c8 < 4 else cwrB
                        row = 32 * (c8 % 4)
                        nc.vector.tensor_copy(out=tgt[row : row + 1, :], in_=cwps[:])
                if c == 1 and msg_prev is not None:
                    node_chain(steps[k - 1], msg_prev, mds.pop(k - 1))
                    msg_prev = None
                if c == 2 and pendB is not None:
                    tailB(pendB)
                    pendB = None

            if msg_prev is not None:
                node_chain(steps[k - 1], msg_prev, mds.pop(k - 1))
                msg_prev = None
            if pendB is not None:
                tailB(pendB)
                pendB = None

            if k - 1 >= 0:
                CWJI = wp.tile([N, N], f32, tag="CWJI", name="CWJI", bufs=2)
                nc.sync.dma_start(out=CWJI[0:32, :], in_=cwrA[::32, :])
                nc.gpsimd.dma_start(out=CWJI[32:64, :], in_=cwrB[::32, :])
                del ems[k - 1]

            # md for step k (consumed by node_chain next tick)
            if in_range and l < L - 1:
                mds[k] = md_chain(steps[k])
                ems[k] = em
            elif in_range:
                ems[k] = em
            if in_range:
                del pps[k]

            if prev is not None:
                pendB = tailA(prev)
                prev = None
            if k - 1 >= 0:
                prev_cwji = (pg, pl, CWJI)

        # drain: node of last step, remaining tails
        if NS - 1 in mds:
            pgd, pld = steps[NS - 1]
            msg_d = wp.tile([H, N], f32r, tag="msg", name="msg_d", bufs=2)
            nc.vector.tensor_reduce(out=msg_d[:],
                                    in_=ems[NS - 1][:].rearrange("p (j i) -> p j i", j=N, i=N),
                                    axis=AX, op=add)
            node_chain(steps[NS - 1], msg_d, mds.pop(NS - 1))
        if pendB is not None:
            tailB(pendB)
            pendB = None
        cg, cl, cwji = prev_cwji
        CWT = wp.tile([N, N], f32, tag="CWT", name="CWT_f", bufs=2)
        for bi in range(2):
            for bj in range(2):
                nc.vector.transpose(out=CWT[bj * 32 : bj * 32 + 32, bi * 32 : bi * 32 + 32],
                                    in_=cwji[bi * 32 : bi * 32 + 32, bj * 32 : bj * 32 + 32])
        tailB(tailA((cg, cl, CWT)))

    nc.compile()
    return nc


def _get_nc(n_graphs, num_devices):
    key = ("v11", n_graphs, num_devices)
    if key not in _CACHE:
        _CACHE[key] = _build(n_graphs, num_devices)
    return _CACHE[key]


def make_in_maps(inputs, n_graphs=GPC, ncores=NCORES):
    consts = _prep_consts(inputs)
    x = np.asarray(inputs["x"], np.float32)
    in_maps = []
    for c in range(ncores):
        xs = x[c * n_graphs : (c + 1) * n_graphs].reshape(n_graphs, N, C)
        mm = dict(consts)
        mm["xin"] = np.ascontiguousarray(xs)
        mm["xtin"] = np.ascontiguousarray(xs.transpose(0, 2, 1))
        in_maps.append(mm)
    return in_maps


def kernel(**inputs) -> np.ndarray:
    from concourse.bass_utils import run_bass_kernel_spmd

    nc = _get_nc(GPC, NCORES)
    in_maps = make_in_maps(inputs)
    res = run_bass_kernel_spmd(nc, in_maps, core_ids=list(range(NCORES)), trace=False)
    outs = [res.results[c]["y"].reshape(GPC, N * C) for c in range(NCORES)]
    return np.concatenate(outs, axis=0).astype(np.float32)


# revision 17
# speedup vs baseline: 1.0061x; 1.0061x over previous
"""Trainium2 Bass kernel v9 for nn_CorrectorEGNN (B=128 graphs, N=64, H=128, L=4).

v6 + single-matmul stage1: dst-major 512-col chunks touch only 8 dst blocks,
so one K=128 select matrix packs src one-hots (rows 0-63), the dynamic
p_i*p_j rows (64-66, written in-place by Pool each step), and dst one-hots
for blocks 0..60 (rows 67-127).  Blocks 61-63 get their B' term from a tiny
K=3 patch matmul on the last 192 columns.  This removes the 2x-cost psum
accumulate pairs that dominated stage1.

Diagonal-edge MLP (md) computed one tick early; its subtraction is folded
into the node MLP as a third accumulating matmul with negated nw1b.
"""

import sys

sys.path.insert(0, "/opt/trn_rl_repo")

import numpy as np

N = 64
C = 3
H = 128
L = 4
B = 128
NCORES = 8
GPC = B // NCORES
E = N * N

_CACHE = {}


def _prep_consts(inputs):
    f32 = np.float32
    ew1 = np.asarray(inputs["edge_w1"], f32)
    d = {}
    d["w1ab"] = np.concatenate(
        [np.concatenate([ew1[l, :H], ew1[l, H : 2 * H]], axis=1) for l in range(L)],
        axis=1,
    )
    wrow = ew1[:, 2 * H]
    d["wrep"] = np.concatenate(
        [np.tile(wrow[l][None, :], (N, 1)) for l in range(L)], axis=1
    )
    d["w3n"] = np.concatenate(
        [np.tile((-2.0 * wrow[l])[None, :], (C, 1)) for l in range(L)], axis=1
    )
    d["w2"] = np.concatenate([np.asarray(inputs["edge_w2"], f32)[l] for l in range(L)], axis=1)
    d["cw1"] = np.concatenate([np.asarray(inputs["coord_w1"], f32)[l] for l in range(L)], axis=1)
    d["cw2c"] = np.concatenate([np.asarray(inputs["coord_w2"], f32)[l] for l in range(L)], axis=1)
    nw1 = np.asarray(inputs["node_w1"], f32)
    d["nw1a"] = np.concatenate([nw1[l, :H] for l in range(L)], axis=1)
    d["nw1b"] = np.concatenate([nw1[l, H:] for l in range(L)], axis=1)
    d["nw1bn"] = -d["nw1b"]
    d["nw2"] = np.concatenate([np.asarray(inputs["node_w2"], f32)[l] for l in range(L)], axis=1)
    bias_cols = []
    for nm in ("edge_b1", "edge_b2", "coord_b1", "node_b1", "node_b2"):
        arr = np.asarray(inputs[nm], f32)
        for l in range(L):
            bias_cols.append(arr[l][:, None])
    d["biases"] = np.concatenate(bias_cols, axis=1)
    d["nerep"] = np.tile(np.asarray(inputs["node_embed"], f32).T, (1, N))
    d["ident"] = np.eye(N, dtype=f32)
    os_val = float(np.asarray(inputs["output_scale"], f32)[0])
    msc = np.zeros((N, 2), f32)
    msc[:, 0] = 1.0
    msc[:, 1] = os_val
    d["msc"] = msc
    d["inv64"] = np.full((1, N), 1.0 / N, f32)
    # unified stage1 select matrix, dst-major e = j*64+i:
    # rows 0-63 src one-hot; rows 64-66 zero (PP written at runtime);
    # rows 67-127 dst one-hot for j=0..60
    S = np.zeros((2 * N, E), f32)
    ee = np.arange(E)
    S[ee % N, ee] = 1.0
    jj = ee // N
    m61 = jj < 61
    S[67 + jj[m61], ee[m61]] = 1.0
    d["s128"] = S
    # patch for dst blocks 61-63 (last 192 columns)
    Sb3 = np.zeros((C, 3 * N), f32)
    cc = np.arange(3 * N)
    Sb3[cc // N, cc] = 1.0
    d["sb3"] = Sb3
    Sd = np.zeros((2 * N, N), f32)
    nn = np.arange(N)
    Sd[nn, nn] = 1.0
    Sd[N + nn, nn] = 1.0
    d["Sdiag"] = Sd
    return d


def _build(n_graphs, num_devices):
    import concourse.bacc as bacc
    import concourse.tile as tile
    import concourse.mybir as mybir
    from contextlib import ExitStack

    dt = mybir.dt
    f32 = dt.float32
    f32r = dt.float32r
    Silu = mybir.ActivationFunctionType.Silu
    add = mybir.AluOpType.add
    sub = mybir.AluOpType.subtract
    mult = mybir.AluOpType.mult
    AX = mybir.AxisListType.X

    nc = bacc.Bacc("TRN2", num_devices=num_devices, enable_partition_id=False)

    dr = {}
    for name, shape in [
        ("xin", [n_graphs, N, C]),
        ("xtin", [n_graphs, C, N]),
        ("s128", [2 * N, E]),
        ("sb3", [C, 3 * N]),
        ("Sdiag", [2 * N, N]),
        ("w1ab", [H, L * 2 * H]),
        ("wrep", [N, L * H]),
        ("w3n", [C, L * H]),
        ("w2", [H, L * H]),
        ("cw1", [H, L * H]),
        ("cw2c", [H, L]),
        ("nw1a", [H, L * H]),
        ("nw1b", [H, L * H]),
        ("nw1bn", [H, L * H]),
        ("nw2", [H, L * H]),
        ("biases", [H, 5 * L]),
        ("nerep", [H, N]),
        ("ident", [N, N]),
        ("msc", [N, 2]),
        ("inv64", [1, N]),
    ]:
        dr[name] = nc.dram_tensor(name, shape, f32, kind="ExternalInput").ap()
    y = nc.dram_tensor("y", [n_graphs, N, C], f32, kind="ExternalOutput").ap()

    F32R_CONSTS = {"s128", "sb3", "Sdiag", "w1ab", "w3n", "w2", "cw1", "cw2c",
                   "nw1a", "nw1b", "nw1bn", "nw2"}

    with nc.allow_low_precision(reason="fp32r matmul inputs"), tile.TileContext(nc) as tc, ExitStack() as es:
        cp = es.enter_context(tc.tile_pool(name="const", bufs=1))
        sp = es.enter_context(tc.tile_pool(name="state", bufs=1))
        wp = es.enter_context(tc.tile_pool(name="work", bufs=2))
        pbig = es.enter_context(tc.tile_pool(name="pbig", bufs=3, space="PSUM"))
        psm = es.enter_context(tc.tile_pool(name="psm", bufs=2, space="PSUM"))

        ct = {}
        for name in ("sb3", "Sdiag", "w1ab", "wrep", "w3n", "w2", "cw1", "cw2c",
                     "nw1a", "nw1b", "nw1bn", "nw2", "biases", "nerep", "ident",
                     "msc", "inv64"):
            cdt = f32r if name in F32R_CONSTS else f32
            t = cp.tile(list(dr[name].shape), cdt, tag=f"c_{name}", name=f"c_{name}")
            nc.sync.dma_start(out=t, in_=dr[name].bitcast(cdt) if cdt is f32r else dr[name])
            ct[name] = t

        def wsl(name, l):
            return ct[name][:, l * H : (l + 1) * H]

        def bsl(bi, l):
            return ct["biases"][:, bi * L + l : bi * L + l + 1]

        HTs, Pxs, PTs = [], [], []
        for g in range(n_graphs):
            HT = sp.tile([H, 2 * N], f32r, tag=f"HT{g}", name=f"HT{g}")
            nc.vector.tensor_copy(out=HT[:, 0:N], in_=ct["nerep"][:].bitcast(f32r))
            PxA = sp.tile([N, 4], f32, tag=f"PxA{g}", name=f"PxA{g}")
            P0 = sp.tile([N, C], f32, tag=f"P0{g}", name=f"P0{g}")
            nc.sync.dma_start(out=PxA[:, 0:3], in_=dr["xin"][g])
            nc.sync.dma_start(out=P0[:], in_=dr["xin"][g])
            nc.vector.memset(PxA[:, 3:4], 1.0)
            Px = [PxA, None, P0]
            PT = sp.tile([C, N], f32r, tag=f"PT{g}", name=f"PT{g}")
            nc.sync.dma_start(out=PT[:], in_=dr["xtin"][g].bitcast(f32r))
            HTs.append(HT)
            Pxs.append(Px)
            PTs.append(PT)
        for g in range(n_graphs):
            PxB = sp.tile([N, 4], f32, tag=f"PxB{g}", name=f"PxB{g}")
            nc.vector.memset(PxB[:, 3:4], 1.0)
            Pxs[g][1] = PxB

        # two rotating select-matrix slots; const rows loaded once into both
        s128_slots = []
        for si in range(2):
            st = wp.tile([2 * N, E], f32r, tag="s128", name=f"s128_{si}")
            nc.sync.dma_start(out=st, in_=dr["s128"].bitcast(f32r))
            s128_slots.append(st)

        def ptile(g, l):  # position tile holding the state entering layer l
            return Pxs[g][l % 2]

        def hcol(l):  # column of the h state entering layer l
            return N * (l % 2)

        lSs = [None] * n_graphs      # [A'(64); B'(64)] per graph (for md)
        gdTs = [None] * n_graphs     # |p|^2 as a row, for the md diag correction
        lS128s = [None] * n_graphs   # packed stage1 lhsT per graph
        B3s = [None] * n_graphs      # B'[61:64] per graph

        def build_ls(g, l, ab_ap):
            """lSfull + packed lS128 + B3 for (g, l)."""
            px = ptile(g, l)
            sq = wp.tile([N, C], f32, tag="sq", name="sq")
            nc.vector.tensor_tensor(out=sq[:], in0=px[:, 0:3], in1=px[:, 0:3], op=mult)
            gd = wp.tile([N, 1], f32, tag="gd", name="gd")
            nc.vector.tensor_reduce(out=gd[:], in_=sq[:], axis=AX, op=add)
            lS = sp.tile([2 * N, H], f32r, tag=f"lS{g}", name=f"lS{g}", bufs=1)
            wr = ct["wrep"][:, l * H : (l + 1) * H]
            nc.vector.scalar_tensor_tensor(out=lS[0:N, :], in0=wr, scalar=gd[:],
                                           in1=ab_ap[:, 0:H], op0=mult, op1=add)
            nc.vector.scalar_tensor_tensor(out=lS[N:, :], in0=wr, scalar=gd[:],
                                           in1=ab_ap[:, H:], op0=mult, op1=add)
            lSs[g] = lS
            gdT = sp.tile([1, N], f32r, tag=f"gdT{g}", name=f"gdT{g}", bufs=1)
            nc.gpsimd.dma_start(out=gdT[:], in_=gd[:].bitcast(f32r))
            gdTs[g] = gdT
            lX = sp.tile([2 * N, H], f32r, tag=f"lX{g}", name=f"lX{g}", bufs=1)
            nc.vector.tensor_copy(out=lX[0:N, :], in_=lS[0:N, :])
            nc.vector.tensor_copy(out=lX[64:67, :], in_=ct["w3n"][:, l * H : (l + 1) * H])
            nc.gpsimd.dma_start(out=lX[67:128, :], in_=lS[64:125, :])
            b3 = sp.tile([C, H], f32r, tag=f"b3{g}", name=f"b3{g}", bufs=1)
            nc.gpsimd.dma_start(out=b3[:], in_=lS[125:128, :])
            lS128s[g] = lX
            B3s[g] = b3

        def build_pp(g, slot_idx):
            """Write PP rows into select-matrix slot; also ppd (diag squares)."""
            st = s128_slots[slot_idx]
            Pv = st[64:67, :].rearrange("p (j i) -> p j i", j=N, i=N)
            pt = PTs[g]
            in0 = pt[:].unsqueeze(1).to_broadcast([C, N, N])
            in1 = pt[:].unsqueeze(2).to_broadcast([C, N, N])
            nc.gpsimd.tensor_tensor(out=Pv, in0=in0, in1=in1, op=mult)
            return (st,)

        def silu(out_ap, in_ap, bias_ap):
            nc.scalar.activation(out=out_ap, in_=in_ap, func=Silu, bias=bias_ap)

        def stage1_chunk(k_step, st, t1, c):
            g, l = k_step
            lX = lS128s[g]
            b3 = B3s[g]
            bt = pbig.tile([H, 1024], f32, tag="big", name="bt1")
            for q in range(2):
                sl = slice(c * 1024 + q * 512, c * 1024 + (q + 1) * 512)
                po = slice(q * 512, (q + 1) * 512)
                last = c == 3 and q == 1
                nc.tensor.matmul(out=bt[:, po], lhsT=lX[:], rhs=st[:, sl],
                                 start=True, stop=not last)
                if last:
                    nc.tensor.matmul(out=bt[:, 832:1024], lhsT=b3[:],
                                     rhs=ct["sb3"][:], start=False, stop=True)
            silu(t1[:, c * 1024 : (c + 1) * 1024], bt[:], bsl(0, l))

        def md_chain(k_step):
            """Diagonal-edge MLP md (one tick ahead of its node use)."""
            g, l = k_step
            lst = lSs[g]
            dz1 = psm.tile([H, N], f32, tag="sm", name="dz1")
            nc.tensor.matmul(out=dz1[:], lhsT=lst[:], rhs=ct["Sdiag"][:],
                             start=True, stop=False)
            nc.tensor.matmul(out=dz1[:], lhsT=ct["w3n"][0:1, l * H : (l + 1) * H],
                             rhs=gdTs[g][:], start=False, stop=True)
            t1d = wp.tile([H, N], f32r, tag="t1d", name="t1d", bufs=2)
            silu(t1d[:], dz1[:], bsl(0, l))
            dz2 = psm.tile([H, N], f32, tag="sm", name="dz2")
            nc.tensor.matmul(out=dz2[:], lhsT=wsl("w2", l), rhs=t1d[:],
                             start=True, stop=True)
            md = wp.tile([H, N], f32r, tag="md", name="md", bufs=2)
            silu(md[:], dz2[:], bsl(1, l))
            return md

        def node_chain(k_step, em, md):
            """msg reduce + node MLP + h update (md subtraction folded in)."""
            g, l = k_step
            HT = HTs[g]
            ha, hb = hcol(l), hcol(l + 1)
            msg = wp.tile([H, N], f32r, tag="msg", name="msg", bufs=2)
            nc.vector.tensor_reduce(out=msg[:],
                                    in_=em[:].rearrange("p (j i) -> p j i", j=N, i=N),
                                    axis=AX, op=add)
            nps = psm.tile([H, N], f32, tag="sm", name="nps")
            nc.tensor.matmul(out=nps[:], lhsT=wsl("nw1a", l), rhs=HT[:, ha : ha + N],
                             start=True, stop=False)
            nc.tensor.matmul(out=nps[:], lhsT=wsl("nw1b", l), rhs=msg[:],
                             start=False, stop=False)
            nc.tensor.matmul(out=nps[:], lhsT=wsl("nw1bn", l), rhs=md[:],
                             start=False, stop=True)
            u = wp.tile([H, N], f32r, tag="u", name="u", bufs=2)
            silu(u[:], nps[:], bsl(3, l))
            nps2 = psm.tile([H, N], f32, tag="sm", name="nps2")
            nc.tensor.matmul(out=nps2[:], lhsT=wsl("nw2", l), rhs=u[:],
                             start=True, stop=True)
            nc.vector.scalar_tensor_tensor(out=HT[:, hb : hb + N], in0=nps2[:],
                                           scalar=bsl(4, l),
                                           in1=HT[:, ha : ha + N], op0=add, op1=add)

        def tailA(prev):
            pg, pl, pCWT = prev
            pa = ptile(pg, pl)
            pb = ptile(pg, pl + 1)
            upd = psm.tile([N, 4], f32, tag="sm", name="upd")
            nc.tensor.matmul(out=upd[:], lhsT=pCWT[:], rhs=pa[:, 0:4],
                             start=True, stop=True)
            tmpP = wp.tile([N, C], f32, tag="tmpP", name="tmpP")
            nc.vector.scalar_tensor_tensor(out=tmpP[:], in0=pa[:, 0:3],
                                           scalar=upd[:, 3:4], in1=upd[:, 0:3],
                                           op0=mult, op1=sub)
            nc.vector.tensor_tensor(out=pb[:, 0:3], in0=pa[:, 0:3],
                                    in1=tmpP[:], op=sub)
            return (pg, pl)

        def tailB(pend):
            pg, pl = pend
            pb_t = ptile(pg, pl + 1)
            pPT = PTs[pg]
            ptp = psm.tile([C, N], f32, tag="sm", name="ptp")
            nc.tensor.transpose(out=ptp[:], in_=pb_t[:, 0:3], identity=ct["ident"][:])
            nc.vector.tensor_copy(out=pPT[:], in_=ptp[:])
            if pl < L - 1:
                hc = hcol(pl + 1)
                ab = psm.tile([N, 2 * H], f32, tag="sm", name="ab")
                nc.tensor.matmul(out=ab[:], lhsT=HTs[pg][:, hc : hc + N],
                                 rhs=ct["w1ab"][:, (pl + 1) * 2 * H : (pl + 2) * 2 * H],
                                 start=True, stop=True)
                build_ls(pg, pl + 1, ab)
            else:
                dxt = wp.tile([N, C], f32, tag="dxt", name="dxt")
                nc.vector.tensor_tensor(out=dxt[:], in0=pb_t[:, 0:3],
                                        in1=Pxs[pg][2][:], op=sub)
                mps = psm.tile([1, C], f32, tag="sm", name="mps")
                nc.tensor.matmul(out=mps[:], lhsT=ct["msc"][:, 0:1], rhs=dxt[:],
                                 start=True, stop=True)
                means = wp.tile([1, C], f32, tag="means", name="means")
                nc.vector.tensor_copy(out=means[:], in_=mps[:])
                mrep = psm.tile([N, C], f32, tag="sm", name="mrep")
                nc.tensor.matmul(out=mrep[:], lhsT=ct["inv64"][:], rhs=means[:],
                                 start=True, stop=True)
                dx2 = wp.tile([N, C], f32, tag="dx2", name="dx2")
                nc.vector.tensor_tensor(out=dx2[:], in0=dxt[:], in1=mrep[:], op=sub)
                dx3 = wp.tile([N, C], f32, tag="dx3", name="dx3")
                nc.vector.tensor_scalar_mul(out=dx3[:], in0=dx2[:], scalar1=ct["msc"][:, 1:2])
                nc.sync.dma_start(out=y[pg], in_=dx3[:])

        # ---- startup ----
        ab0 = psm.tile([N, 2 * H], f32, tag="sm", name="ab0")
        nc.tensor.matmul(out=ab0[:], lhsT=HTs[0][:, 0:N], rhs=ct["w1ab"][:, 0 : 2 * H],
                         start=True, stop=True)
        for g in range(n_graphs):
            build_ls(g, 0, ab0)

        steps = [(g, l) for l in range(L) for g in range(n_graphs)]
        NS = len(steps)
        pps = {0: build_pp(steps[0][0], 0), 1: build_pp(steps[1][0], 1)}
        prev = None        # awaiting tailA (pos update)
        pendB = None       # awaiting tailB
        prev_cwji = None   # awaiting transpose
        mds = {}           # per-step md tiles
        ems = {}           # per-step m tiles

        t1_cur = wp.tile([H, E], f32r, tag="t1", name="t1", bufs=2)
        for c in range(4):
            stage1_chunk(steps[0], pps[0][0], t1_cur, c)

        for k in range(NS):
            g, l = steps[k]
            t1 = t1_cur

            if k + 2 < NS:
                pps[k + 2] = build_pp(steps[k + 2][0], k % 2)

            # CWJI -> CWT transpose for previous step
            if prev_cwji is not None:
                cg, cl, cwji = prev_cwji
                CWT = wp.tile([N, N], f32, tag="CWT", name="CWT", bufs=2)
                for bi in range(2):
                    for bj in range(2):
                        nc.vector.transpose(out=CWT[bj * 32 : bj * 32 + 32, bi * 32 : bi * 32 + 32],
                                            in_=cwji[bi * 32 : bi * 32 + 32, bj * 32 : bj * 32 + 32])
                prev = (cg, cl, CWT)
                prev_cwji = None

            # stage 2 of k woven with stage 1 of k+1
            if k + 1 < NS:
                t1_cur = wp.tile([H, E], f32r, tag="t1", name="t1", bufs=2)
            em = wp.tile([H, E], f32r, tag="m", name="em", bufs=2)
            for c in range(4):
                bt = pbig.tile([H, 1024], f32, tag="big", name="bt2")
                for q in range(2):
                    sl = slice(c * 1024 + q * 512, c * 1024 + (q + 1) * 512)
                    po = slice(q * 512, (q + 1) * 512)
                    nc.tensor.matmul(out=bt[:, po], lhsT=wsl("w2", l), rhs=t1[:, sl],
                                     start=True, stop=True)
                silu(em[:, c * 1024 : (c + 1) * 1024], bt[:], bsl(1, l))
                if k + 1 < NS:
                    stage1_chunk(steps[k + 1], pps[k + 1][0], t1_cur, c)

            # stage 3 + cw2, with node/tailB woven between chunks
            t2 = wp.tile([H, E], f32r, tag="t2", name="t2", bufs=1)
            cwrA = wp.tile([97, 512], f32, tag="cwrA", name="cwrA", bufs=2)
            cwrB = wp.tile([97, 512], f32, tag="cwrB", name="cwrB", bufs=2)
            for c in range(4):
                bt = pbig.tile([H, 1024], f32, tag="big", name="bt3")
                for q in range(2):
                    sl = slice(c * 1024 + q * 512, c * 1024 + (q + 1) * 512)
                    po = slice(q * 512, (q + 1) * 512)
                    nc.tensor.matmul(out=bt[:, po], lhsT=wsl("cw1", l), rhs=em[:, sl],
                                     start=True, stop=True)
                silu(t2[:, c * 1024 : (c + 1) * 1024], bt[:], bsl(2, l))
                for q in range(2):
                    c8 = 2 * c + q
                    cwps = pbig.tile([1, 512], f32, tag="big", name="cwps")
                    nc.tensor.matmul(out=cwps[:], lhsT=ct["cw2c"][:, l : l + 1],
                                     rhs=t2[:, c8 * 512 : (c8 + 1) * 512],
                                     start=True, stop=True)
                    tgt = cwrA if c8 < 4 else cwrB
                    row = 32 * (c8 % 4)
                    nc.vector.tensor_copy(out=tgt[row : row + 1, :], in_=cwps[:])
                if c == 1 and k - 1 >= 0 and k - 1 in mds:
                    node_chain(steps[k - 1], ems[k - 1], mds.pop(k - 1))
                    del ems[k - 1]
                if c == 2 and pendB is not None:
                    tailB(pendB)
                    pendB = None
            if k - 1 >= 0 and k - 1 in mds:
                node_chain(steps[k - 1], ems[k - 1], mds.pop(k - 1))
                del ems[k - 1]
            if pendB is not None:
                tailB(pendB)
                pendB = None
            CWJI = wp.tile([N, N], f32, tag="CWJI", name="CWJI", bufs=2)
            nc.sync.dma_start(out=CWJI[0:32, :], in_=cwrA[::32, :])
            nc.gpsimd.dma_start(out=CWJI[32:64, :], in_=cwrB[::32, :])

            # md for THIS step (consumed by node_chain next tick)
            if l < L - 1:
                mds[k] = md_chain(steps[k])
                ems[k] = em
            del pps[k]

            if prev is not None:
                pendB = tailA(prev)
                prev = None
            prev_cwji = (g, l, CWJI)

        # drain
        if NS - 1 in mds:
            node_chain(steps[NS - 1], ems.pop(NS - 1), mds.pop(NS - 1))
        if pendB is not None:
            tailB(pendB)
            pendB = None
        cg, cl, cwji = prev_cwji
        CWT = wp.tile([N, N], f32, tag="CWT", name="CWT_f", bufs=2)
        for bi in range(2):
            for bj in range(2):
                nc.vector.transpose(out=CWT[bj * 32 : bj * 32 + 32, bi * 32 : bi * 32 + 32],
                                    in_=cwji[bi * 32 : bi * 32 + 32, bj * 32 : bj * 32 + 32])
        tailB(tailA((cg, cl, CWT)))

    nc.compile()
    return nc


def _get_nc(n_graphs, num_devices):
    key = ("v10", n_graphs, num_devices)
    if key not in _CACHE:
        _CACHE[key] = _build(n_graphs, num_devices)
    return _CACHE[key]


def make_in_maps(inputs, n_graphs=GPC, ncores=NCORES):
    consts = _prep_consts(inputs)
    x = np.asarray(inputs["x"], np.float32)
    in_maps = []
    for c in range(ncores):
        xs = x[c * n_graphs : (c + 1) * n_graphs].reshape(n_graphs, N, C)
        mm = dict(consts)
        mm["xin"] = np.ascontiguousarray(xs)
        mm["xtin"] = np.ascontiguousarray(xs.transpose(0, 2, 1))
        in_maps.append(mm)
    return in_maps


def kernel(**inputs) -> np.ndarray:
    from concourse.bass_utils import run_bass_kernel_spmd

    nc = _get_nc(GPC, NCORES)
    in_maps = make_in_maps(inputs)
    res = run_bass_kernel_spmd(nc, in_maps, core_ids=list(range(NCORES)), trace=False)
    outs = [res.results[c]["y"].reshape(GPC, N * C) for c in range(NCORES)]
    return np.concatenate(outs, axis=0).astype(np.float32)
